# revision 9
# baseline (speedup 1.0000x reference)
"""GRU decoder kernel for Trainium2 (8 NeuronCores, data-parallel over batch).

Math (PyTorch GRU, gate order r,z,n), per batch element:
    gx_t = x_t * w_ih + b_ih              (input dim == 1 -> rank-1)
    gh_t = h_{t-1} @ w_hh.T + b_hh
    r = sigmoid(gx_r + gh_r); z = sigmoid(gx_z + gh_z)
    n = tanh(gx_n + b_ih_n + r * (gh_n + b_hh_n))
    h_t = (1-z)*n + z*h_{t-1}
    out = h_T @ fc_w.T + fc_b

v2 layout (per core, B_c = 1024 batch):
  - partition-stacked: batch 0-511 ("u") on SBUF partitions 0-63,
    batch 512-1023 ("v") on partitions 64-127 for H and all gate tensors.
  - h-matmuls: block-diag lhsT diag(Wg.T, Wg.T) [128,128], K=128 covering
    both halves in one pass per gate.
  - x + bias contribution: X4 tile interleaves (u-x, v-x, ones, pad) with
    period 4 on partitions; per step a K=32 matmul with a mostly-zero
    [32,128] weight slice adds w_g*x + b_g for both halves (biases ride
    the ones-row -> bias-free merged sigmoid).
  - PSUM bank packing: bankRZ [128,512] = r|z pre-acts -> ONE merged
    sigmoid; bankNX [128,512] = hn|xn.
  - xn drained to fp16 by Scalar engine so the T2 add runs at DVE 2x.
  - U = h - n runs on GpSimd (otherwise idle engine).
  - NGROUP phase-shifted batch groups pipeline the serial chain.
"""

import os
import sys

sys.path.insert(0, "/opt/trn_rl_repo")

import numpy as np
from contextlib import ExitStack

HIDDEN = 64
OUT = 256
B = 8192
T = int(os.environ.get("GRU_T", 1024))
NCORES = 8
BC = B // NCORES          # 1024 batch per core
HB = BC // 2              # 512 batch per partition-half
UNROLL = 32               # steps per block (4 partitions per step in X4)
NGROUP = int(os.environ.get("GRU_NGROUP", 2))  # phase-shifted batch groups
NBLK = T // UNROLL        # number of blocks

_CACHE = {}


def _build():
    import concourse.bass as bass
    import concourse.tile as tile
    from concourse import bacc, mybir

    f16 = mybir.dt.float16
    f32 = mybir.dt.float32
    AF = mybir.ActivationFunctionType
    OP = mybir.AluOpType

    nc = bacc.Bacc("TRN2", target_bir_lowering=False, debug=False,
                   num_devices=NCORES)

    d_x = nc.dram_tensor("xt", [128, NBLK, HB], f16, kind="ExternalInput").ap()
    d_dr = nc.dram_tensor("dr", [128, 128], f16, kind="ExternalInput").ap()
    d_dz = nc.dram_tensor("dz", [128, 128], f16, kind="ExternalInput").ap()
    d_dn = nc.dram_tensor("dn", [128, 128], f16, kind="ExternalInput").ap()
    d_xwr = nc.dram_tensor("xwr", [128, 8 * 128], f16, kind="ExternalInput").ap()
    d_xwz = nc.dram_tensor("xwz", [128, 8 * 128], f16, kind="ExternalInput").ap()
    d_xwn = nc.dram_tensor("xwn", [128, 8 * 128], f16, kind="ExternalInput").ap()
    d_bnh = nc.dram_tensor("bnh", [128, 1], f32, kind="ExternalInput").ap()
    d_fcw = nc.dram_tensor("fcw", [128, OUT], f16, kind="ExternalInput").ap()
    d_fcb = nc.dram_tensor("fcb", [128, 2], f32, kind="ExternalInput").ap()
    d_out = nc.dram_tensor("out", [OUT, BC], f32, kind="ExternalOutput").ap()

    with tile.TileContext(nc) as tc, ExitStack() as ctx:
        singles = ctx.enter_context(tc.tile_pool(name="singles", bufs=1))
        work = ctx.enter_context(tc.tile_pool(name="work", bufs=2))
        psum = ctx.enter_context(tc.tile_pool(name="psum", bufs=1, space="PSUM"))

        X = singles.tile([128, NBLK, HB], f16)
        DR = singles.tile([128, 128], f16)
        DZ = singles.tile([128, 128], f16)
        DN = singles.tile([128, 128], f16)
        XWR = singles.tile([128, 8 * 128], f16)
        XWZ = singles.tile([128, 8 * 128], f16)
        XWN = singles.tile([128, 8 * 128], f16)
        BNH = singles.tile([128, 1], f32)
        FCW = singles.tile([128, OUT], f16)
        FCB = singles.tile([128, 2], f32)
        HG = HB // NGROUP   # free-dim width per pipelined batch group
        # one H tile per group: groups must not share a tile or the
        # dependency tracker serializes their chains
        Hs = [singles.tile([128, HG], f16, name=f"H{g}")
              for g in range(NGROUP)]

        for dst, src in ((X, d_x), (DR, d_dr), (DZ, d_dz), (DN, d_dn),
                         (XWR, d_xwr), (XWZ, d_xwz), (XWN, d_xwn),
                         (BNH, d_bnh), (FCW, d_fcw), (FCB, d_fcb)):
            nc.gpsimd.dma_start(dst[:], src[:])
        for Hg in Hs:
            nc.vector.memset(Hg[:], 0.0)

        def step(q, blk, g):
            fd = slice(g * HG, (g + 1) * HG)
            H = Hs[g]
            strip = 32 * (q // 8)
            qq = q % 8
            ksl = slice(strip, strip + 32)
            wsl = slice(qq * 128, (qq + 1) * 128)
            xs = X[ksl, blk, fd]

            bankRZ = psum.tile([128, 2 * HG], f32, tag=f"bankRZ{g}")
            bankNX = psum.tile([128, 2 * HG], f32, tag=f"bankNX{g}")
            rr = slice(0, HG)
            zz = slice(HG, 2 * HG)
            # r and z pre-activations, both halves per pass; biases come in
            # via the ones-row of the X weight slices.
            xtp = (strip, 0)
            nc.tensor.matmul(bankRZ[:, rr], DR[:], H[:],
                             start=True, stop=False)
            nc.tensor.matmul(bankRZ[:, rr], XWR[ksl, wsl], xs,
                             start=False, stop=True, tile_position=xtp)
            nc.tensor.matmul(bankRZ[:, zz], DZ[:], H[:],
                             start=True, stop=False)
            nc.tensor.matmul(bankRZ[:, zz], XWZ[ksl, wsl], xs,
                             start=False, stop=True, tile_position=xtp)
            nc.tensor.matmul(bankNX[:, rr], DN[:], H[:],
                             start=True, stop=True)
            nc.tensor.matmul(bankNX[:, zz], XWN[ksl, wsl], xs,
                             start=True, stop=True, tile_position=xtp)

            SRZ = work.tile([128, 2 * HG], f16, tag=f"SRZ{g}")
            XN = work.tile([128, HG], f16, tag=f"XN{g}")
            T1 = work.tile([128, HG], f16, tag=f"T1{g}")
            T2 = work.tile([128, HG], f16, tag=f"T2{g}")
            NN = work.tile([128, HG], f16, tag=f"NN{g}")
            U = work.tile([128, HG], f16, tag=f"U{g}")
            V = work.tile([128, HG], f16, tag=f"V{g}")
            nc.scalar.activation(SRZ[:], bankRZ[:], AF.Sigmoid)
            nc.scalar.activation(XN[:], bankNX[:, zz], AF.Identity)
            # T1 = (hn + b_hh_n) * r
            nc.vector.scalar_tensor_tensor(T1[:], bankNX[:, rr], BNH[:],
                                           SRZ[:, rr], op0=OP.add, op1=OP.mult)
            nc.vector.tensor_add(T2[:], T1[:], XN[:])
            nc.scalar.activation(NN[:], T2[:], AF.Tanh)
            # h' = n + z*(h - n)
            nc.gpsimd.tensor_sub(U[:], H[:], NN[:])
            nc.vector.tensor_mul(V[:], SRZ[:, zz], U[:])
            nc.vector.tensor_add(H[:], NN[:], V[:])

        def body(blk):
            for q in range(UNROLL):
                for g in range(NGROUP):
                    step(q, blk, g)

        if NBLK == 1:
            body(0)
        else:
            with tc.For_i(0, NBLK, 1,
                          hint_engines=(mybir.EngineType.PE,)) as i:
                body(bass.ds(i, 1))

        # Final FC: out[o, b] = sum_k fc_w[o, k] h[b, k] + fc_b[o]
        for oh in range(2):
            osl = slice(oh * 128, (oh + 1) * 128)
            for g in range(NGROUP):
                H = Hs[g]
                fc_u = psum.tile([128, HG], f32, tag="bankRZ0")
                fc_v = psum.tile([128, HG], f32, tag="bankNX0")
                nc.tensor.matmul(fc_u[:], FCW[0:64, osl], H[0:64, :],
                                 start=True, stop=True, tile_position=(0, 0))
                nc.tensor.matmul(fc_v[:], FCW[64:128, osl], H[64:128, :],
                                 start=True, stop=True, tile_position=(64, 0))
                Ou = work.tile([128, HG], f32, tag="Ou")
                Ov = work.tile([128, HG], f32, tag="Ov")
                nc.scalar.activation(Ou[:], fc_u[:], AF.Identity,
                                     bias=FCB[:, oh:oh + 1])
                nc.scalar.activation(Ov[:], fc_v[:], AF.Identity,
                                     bias=FCB[:, oh:oh + 1])
                gd = slice(g * HG, (g + 1) * HG)
                gdv = slice(HB + g * HG, HB + (g + 1) * HG)
                nc.gpsimd.dma_start(d_out[osl, gd], Ou[:])
                nc.gpsimd.dma_start(d_out[osl, gdv], Ov[:])

    nc.compile()
    return nc


def _host_inputs(x, w_ih, w_hh, b_ih, b_hh, fc_w, fc_b):
    """Build the per-core in_maps (numpy, laid out exactly as SBUF tiles)."""
    f16 = np.float16
    f32 = np.float32
    x = np.asarray(x, f32)
    w_ih = np.asarray(w_ih, f32)
    w_hh = np.asarray(w_hh, f32)
    b_ih = np.asarray(b_ih, f32)
    b_hh = np.asarray(b_hh, f32)
    fc_w = np.asarray(fc_w, f32)
    fc_b = np.asarray(fc_b, f32)

    def diag2(seg):
        t = w_hh[seg, :].T                      # [64(k), 64(m)]
        d = np.zeros((128, 128), f32)
        d[0:64, 0:64] = t
        d[64:128, 64:128] = t
        return d.astype(f16)

    def xw(seg, bias):
        # [32, 8, 128]: row 4*qq+r within a strip, step-in-strip qq
        w = w_ih[seg, 0]                        # [64]
        b = bias                                # [64]
        m = np.zeros((32, 8, 128), f32)
        for qq in range(8):
            m[4 * qq + 0, qq, 0:64] = w
            m[4 * qq + 1, qq, 64:128] = w
            m[4 * qq + 2, qq, 0:64] = b
            m[4 * qq + 2, qq, 64:128] = b
        m = m.reshape(32, 8 * 128)
        return np.tile(m, (4, 1)).astype(f16)   # [128, 1024] (4 strips)

    shared = {
        "dr": diag2(slice(0, 64)),
        "dz": diag2(slice(64, 128)),
        "dn": diag2(slice(128, 192)),
        "xwr": xw(slice(0, 64), b_ih[0:64] + b_hh[0:64]),
        "xwz": xw(slice(64, 128), b_ih[64:128] + b_hh[64:128]),
        "xwn": xw(slice(128, 192), b_ih[128:192]),
        "bnh": np.tile(b_hh[128:192].reshape(-1, 1), (2, 1)).astype(f32),
        "fcw": np.vstack([fc_w.T, fc_w.T]).astype(f16),  # [128, 256]
        "fcb": np.stack([fc_b[0:128], fc_b[128:256]], 1).astype(f32),
    }

    in_maps = []
    for c in range(NCORES):
        xs = x[c * BC:(c + 1) * BC, :T, 0]            # [BC b, T t]
        xT = np.ascontiguousarray(xs.T)               # [T, BC]
        xr = xT.reshape(NBLK, UNROLL, BC)             # [blk, q, b]
        X4 = np.zeros((128, NBLK, HB), f32)
        qs = np.arange(UNROLL)
        X4[4 * qs + 0, :, :] = xr[:, :, 0:HB].transpose(1, 0, 2)
        X4[4 * qs + 1, :, :] = xr[:, :, HB:BC].transpose(1, 0, 2)
        X4[4 * qs + 2, :, :] = 1.0
        m = dict(shared)
        m["xt"] = X4.astype(f16)
        in_maps.append(m)
    return in_maps


def _run(in_maps, trace=False):
    from concourse import bass_utils
    if "nc" not in _CACHE:
        _CACHE["nc"] = _build()
    nc = _CACHE["nc"]
    res = bass_utils.run_bass_kernel_spmd(
        nc, in_maps, core_ids=list(range(NCORES)), trace=trace)
    return res


def kernel(**inputs):
    in_maps = _host_inputs(**inputs)
    res = _run(in_maps, trace=False)
    out = np.empty([B, OUT], np.float32)
    for c in range(NCORES):
        out[c * BC:(c + 1) * BC, :] = res.results[c]["out"].T
    return out


# revision 11
# speedup vs baseline: 1.3789x; 1.3789x over previous
"""GRU decoder kernel for Trainium2 (8 NeuronCores, data-parallel over batch).

Math (PyTorch GRU, gate order r,z,n), per batch element:
    gx_t = x_t * w_ih + b_ih              (input dim == 1 -> rank-1)
    gh_t = h_{t-1} @ w_hh.T + b_hh
    r = sigmoid(gx_r + gh_r); z = sigmoid(gx_z + gh_z)
    n = tanh(gx_n + b_ih_n + r * (gh_n + b_hh_n))
    h_t = (1-z)*n + z*h_{t-1}
    out = h_T @ fc_w.T + fc_b

v2 layout (per core, B_c = 1024 batch):
  - partition-stacked: batch 0-511 ("u") on SBUF partitions 0-63,
    batch 512-1023 ("v") on partitions 64-127 for H and all gate tensors.
  - h-matmuls: block-diag lhsT diag(Wg.T, Wg.T) [128,128], K=128 covering
    both halves in one pass per gate.
  - x + bias contribution: X4 tile interleaves (u-x, v-x, ones, pad) with
    period 4 on partitions; per step a K=32 matmul with a mostly-zero
    [32,128] weight slice adds w_g*x + b_g for both halves (biases ride
    the ones-row -> bias-free merged sigmoid).
  - PSUM bank packing: bankRZ [128,512] = r|z pre-acts -> ONE merged
    sigmoid; bankNX [128,512] = hn|xn.
  - xn drained to fp16 by Scalar engine so the T2 add runs at DVE 2x.
  - U = h - n runs on GpSimd (otherwise idle engine).
  - NGROUP phase-shifted batch groups pipeline the serial chain.
"""

import os
import sys

sys.path.insert(0, "/opt/trn_rl_repo")

import numpy as np
from contextlib import ExitStack

HIDDEN = 64
OUT = 256
B = 8192
T = int(os.environ.get("GRU_T", 1024))
NCORES = 8
BC = B // NCORES          # 1024 batch per core
HB = BC // 2              # 512 batch per partition-half
UNROLL = 32               # steps per block (4 partitions per step in X4)
NGROUP = int(os.environ.get("GRU_NGROUP", 2))  # phase-shifted batch groups
NBLK = T // UNROLL        # number of blocks

_CACHE = {}


def _build():
    import concourse.bass as bass
    import concourse.tile as tile
    from concourse import bacc, mybir

    f16 = mybir.dt.float16
    f32 = mybir.dt.float32
    AF = mybir.ActivationFunctionType
    OP = mybir.AluOpType

    nc = bacc.Bacc("TRN2", target_bir_lowering=False, debug=False,
                   num_devices=NCORES)

    d_x = nc.dram_tensor("xt", [128, NBLK, HB], f16, kind="ExternalInput").ap()
    d_dr = nc.dram_tensor("dr", [128, 128], f16, kind="ExternalInput").ap()
    d_dz = nc.dram_tensor("dz", [128, 128], f16, kind="ExternalInput").ap()
    d_dn = nc.dram_tensor("dn", [128, 128], f16, kind="ExternalInput").ap()
    d_xwr = nc.dram_tensor("xwr", [128, 8 * 128], f16, kind="ExternalInput").ap()
    d_xwz = nc.dram_tensor("xwz", [128, 8 * 128], f16, kind="ExternalInput").ap()
    d_xwn = nc.dram_tensor("xwn", [128, 8 * 128], f16, kind="ExternalInput").ap()
    d_bnh = nc.dram_tensor("bnh", [128, 1], f32, kind="ExternalInput").ap()
    d_fcw = nc.dram_tensor("fcw", [128, OUT], f16, kind="ExternalInput").ap()
    d_fcb = nc.dram_tensor("fcb", [128, 2], f32, kind="ExternalInput").ap()
    d_out = nc.dram_tensor("out", [OUT, BC], f32, kind="ExternalOutput").ap()

    with tile.TileContext(nc) as tc, ExitStack() as ctx:
        singles = ctx.enter_context(tc.tile_pool(name="singles", bufs=1))
        work = ctx.enter_context(tc.tile_pool(name="work", bufs=2))
        psum = ctx.enter_context(tc.tile_pool(name="psum", bufs=1, space="PSUM"))

        X = singles.tile([128, NBLK, HB], f16)
        DR = singles.tile([128, 128], f16)
        DZ = singles.tile([128, 128], f16)
        DN = singles.tile([128, 128], f16)
        XWR = singles.tile([128, 8 * 128], f16)
        XWZ = singles.tile([128, 8 * 128], f16)
        XWN = singles.tile([128, 8 * 128], f16)
        BNH = singles.tile([128, 1], f32)
        FCW = singles.tile([128, OUT], f16)
        FCB = singles.tile([128, 2], f32)
        HG = HB // NGROUP   # free-dim width per pipelined batch group
        # one H tile per group: groups must not share a tile or the
        # dependency tracker serializes their chains
        Hs = [singles.tile([128, HG], f16, name=f"H{g}")
              for g in range(NGROUP)]

        for dst, src in ((X, d_x), (DR, d_dr), (DZ, d_dz), (DN, d_dn),
                         (XWR, d_xwr), (XWZ, d_xwz), (XWN, d_xwn),
                         (BNH, d_bnh), (FCW, d_fcw), (FCB, d_fcb)):
            nc.gpsimd.dma_start(dst[:], src[:])
        for Hg in Hs:
            nc.vector.memset(Hg[:], 0.0)

        def step(q, blk, g):
            fd = slice(g * HG, (g + 1) * HG)
            H = Hs[g]
            strip = 32 * (q // 8)
            qq = q % 8
            ksl = slice(strip, strip + 32)
            wsl = slice(qq * 128, (qq + 1) * 128)
            xs = X[ksl, blk, fd]

            bankRZ = psum.tile([128, 2 * HG], f32, tag=f"bankRZ{g}")
            bankNX = psum.tile([128, 2 * HG], f32, tag=f"bankNX{g}")
            rr = slice(0, HG)
            zz = slice(HG, 2 * HG)
            # r and z pre-activations, both halves per pass; biases come in
            # via the ones-row of the X weight slices.
            xtp = (strip, 0)
            # Each PSUM region keeps a contiguous start/stop pair (interleaved
            # accumulation groups within a bank mis-accumulate).  x-matmuls
            # lead their region so they can run during the previous step's
            # elementwise phase; h-matmuls close each group on the chain.
            # Region order: r first (feeds sigmoid), n second (feeds stt),
            # z last (feeds the late z*(h-n) multiply).
            nc.tensor.matmul(bankRZ[:, rr], XWR[ksl, wsl], xs,
                             start=True, stop=False, tile_position=xtp)
            nc.tensor.matmul(bankRZ[:, rr], DR[:], H[:],
                             start=False, stop=True)
            nc.tensor.matmul(bankNX[:, zz], XWN[ksl, wsl], xs,
                             start=True, stop=True, tile_position=xtp)
            nc.tensor.matmul(bankNX[:, rr], DN[:], H[:],
                             start=True, stop=True)
            nc.tensor.matmul(bankRZ[:, zz], XWZ[ksl, wsl], xs,
                             start=True, stop=False, tile_position=xtp)
            nc.tensor.matmul(bankRZ[:, zz], DZ[:], H[:],
                             start=False, stop=True)

            SR = work.tile([128, HG], f16, tag=f"SR{g}")
            SZ = work.tile([128, HG], f16, tag=f"SZ{g}")
            XN = work.tile([128, HG], f16, tag=f"XN{g}")
            T1 = work.tile([128, HG], f16, tag=f"T1{g}")
            T2 = work.tile([128, HG], f16, tag=f"T2{g}")
            NN = work.tile([128, HG], f16, tag=f"NN{g}")
            U = work.tile([128, HG], f16, tag=f"U{g}")
            V = work.tile([128, HG], f16, tag=f"V{g}")
            nc.scalar.activation(SR[:], bankRZ[:, rr], AF.Sigmoid)
            nc.scalar.activation(XN[:], bankNX[:, zz], AF.Identity)
            nc.scalar.activation(SZ[:], bankRZ[:, zz], AF.Sigmoid)
            # T1 = (hn + b_hh_n) * r
            nc.vector.scalar_tensor_tensor(T1[:], bankNX[:, rr], BNH[:],
                                           SR[:], op0=OP.add, op1=OP.mult)
            nc.vector.tensor_add(T2[:], T1[:], XN[:])
            nc.scalar.activation(NN[:], T2[:], AF.Tanh)
            # h' = n + z*(h - n)
            nc.vector.tensor_sub(U[:], H[:], NN[:])
            nc.vector.tensor_mul(V[:], SZ[:], U[:])
            nc.vector.tensor_add(H[:], NN[:], V[:])

        def body(blk):
            for q in range(UNROLL):
                for g in range(NGROUP):
                    step(q, blk, g)

        if NBLK == 1:
            body(0)
        else:
            with tc.For_i(0, NBLK, 1,
                          hint_engines=(mybir.EngineType.PE,)) as i:
                body(bass.ds(i, 1))

        # Final FC: out[o, b] = sum_k fc_w[o, k] h[b, k] + fc_b[o]
        for oh in range(2):
            osl = slice(oh * 128, (oh + 1) * 128)
            for g in range(NGROUP):
                H = Hs[g]
                fc_u = psum.tile([128, HG], f32, tag="bankRZ0")
                fc_v = psum.tile([128, HG], f32, tag="bankNX0")
                nc.tensor.matmul(fc_u[:], FCW[0:64, osl], H[0:64, :],
                                 start=True, stop=True, tile_position=(0, 0))
                nc.tensor.matmul(fc_v[:], FCW[64:128, osl], H[64:128, :],
                                 start=True, stop=True, tile_position=(64, 0))
                Ou = work.tile([128, HG], f32, tag="Ou")
                Ov = work.tile([128, HG], f32, tag="Ov")
                nc.scalar.activation(Ou[:], fc_u[:], AF.Identity,
                                     bias=FCB[:, oh:oh + 1])
                nc.scalar.activation(Ov[:], fc_v[:], AF.Identity,
                                     bias=FCB[:, oh:oh + 1])
                gd = slice(g * HG, (g + 1) * HG)
                gdv = slice(HB + g * HG, HB + (g + 1) * HG)
                nc.gpsimd.dma_start(d_out[osl, gd], Ou[:])
                nc.gpsimd.dma_start(d_out[osl, gdv], Ov[:])

    nc.compile()
    return nc


def _host_inputs(x, w_ih, w_hh, b_ih, b_hh, fc_w, fc_b):
    """Build the per-core in_maps (numpy, laid out exactly as SBUF tiles)."""
    f16 = np.float16
    f32 = np.float32
    x = np.asarray(x, f32)
    w_ih = np.asarray(w_ih, f32)
    w_hh = np.asarray(w_hh, f32)
    b_ih = np.asarray(b_ih, f32)
    b_hh = np.asarray(b_hh, f32)
    fc_w = np.asarray(fc_w, f32)
    fc_b = np.asarray(fc_b, f32)

    def diag2(seg):
        t = w_hh[seg, :].T                      # [64(k), 64(m)]
        d = np.zeros((128, 128), f32)
        d[0:64, 0:64] = t
        d[64:128, 64:128] = t
        return d.astype(f16)

    def xw(seg, bias):
        # [32, 8, 128]: row 4*qq+r within a strip, step-in-strip qq
        w = w_ih[seg, 0]                        # [64]
        b = bias                                # [64]
        m = np.zeros((32, 8, 128), f32)
        for qq in range(8):
            m[4 * qq + 0, qq, 0:64] = w
            m[4 * qq + 1, qq, 64:128] = w
            m[4 * qq + 2, qq, 0:64] = b
            m[4 * qq + 2, qq, 64:128] = b
        m = m.reshape(32, 8 * 128)
        return np.tile(m, (4, 1)).astype(f16)   # [128, 1024] (4 strips)

    shared = {
        "dr": diag2(slice(0, 64)),
        "dz": diag2(slice(64, 128)),
        "dn": diag2(slice(128, 192)),
        "xwr": xw(slice(0, 64), b_ih[0:64] + b_hh[0:64]),
        "xwz": xw(slice(64, 128), b_ih[64:128] + b_hh[64:128]),
        "xwn": xw(slice(128, 192), b_ih[128:192]),
        "bnh": np.tile(b_hh[128:192].reshape(-1, 1), (2, 1)).astype(f32),
        "fcw": np.vstack([fc_w.T, fc_w.T]).astype(f16),  # [128, 256]
        "fcb": np.stack([fc_b[0:128], fc_b[128:256]], 1).astype(f32),
    }

    in_maps = []
    for c in range(NCORES):
        xs = x[c * BC:(c + 1) * BC, :T, 0]            # [BC b, T t]
        xT = np.ascontiguousarray(xs.T)               # [T, BC]
        xr = xT.reshape(NBLK, UNROLL, BC)             # [blk, q, b]
        X4 = np.zeros((128, NBLK, HB), f32)
        qs = np.arange(UNROLL)
        X4[4 * qs + 0, :, :] = xr[:, :, 0:HB].transpose(1, 0, 2)
        X4[4 * qs + 1, :, :] = xr[:, :, HB:BC].transpose(1, 0, 2)
        X4[4 * qs + 2, :, :] = 1.0
        m = dict(shared)
        m["xt"] = X4.astype(f16)
        in_maps.append(m)
    return in_maps


def _run(in_maps, trace=False):
    from concourse import bass_utils
    if "nc" not in _CACHE:
        _CACHE["nc"] = _build()
    nc = _CACHE["nc"]
    res = bass_utils.run_bass_kernel_spmd(
        nc, in_maps, core_ids=list(range(NCORES)), trace=trace)
    return res


def kernel(**inputs):
    in_maps = _host_inputs(**inputs)
    res = _run(in_maps, trace=False)
    out = np.empty([B, OUT], np.float32)
    for c in range(NCORES):
        out[c * BC:(c + 1) * BC, :] = res.results[c]["out"].T
    return out
